# revision 44
# baseline (speedup 1.0000x reference)
"""Trainium2 Bass kernel for GQA attention (b=2, s=2048, dim=1024, 16 q / 4 kv heads).

Sharding: 8 cores = 2 (batch) x 4 (head groups). Each core owns one batch
element and 4 q-heads + 1 kv-head (Wq/Wk/Wv column-sharded, Wo row-sharded).
Host pre-transposes everything contraction-major in bf16; host sums the 4
Wo partials per batch element in fp32.

v3 structure:
  - loops: head-pair hp (outer) -> tq chunk c of 512 -> tk tile j.
  - scores for both heads of a pair in ONE [128, 2, 512] fp32 PSUM tile
    (row-tiled concurrent matmuls at tile_position (0,0)/(64,0)); exp is a
    single ScalarE instruction per (hp, c, j); ScalarE runs ONLY exp.
  - causal diag mask applied ON THE PE: an extra accumulate-matmul adds a
    strictly-lower-triangular -60000 constant (ida stationary, ltri moving)
    into the scores PSUM before exp -> no cross-engine hop in the chain.
  - per-j emission order: exp(j) | fillers | scores(j+1) | PV(j), so the PE
    FIFO always has filler work while exp(j) runs and PV(j) never blocks
    the next scores. scores(c+1,0) is emitted before completion(c).
  - fillers are micro-tasks (2 matmuls each) scheduled so every input is
    produced a chunk ahead of first use and only DMA-landed data is touched.
  - normalization transpose path entirely in bf16 (pvs/pT/rec4/anrm).
"""

import sys
from contextlib import ExitStack

for _p in ("/opt/trn_rl_repo",):
    if _p not in sys.path:
        sys.path.insert(0, _p)

import numpy as np
import ml_dtypes

BF16 = ml_dtypes.bfloat16

P = 128
S = 2048          # sequence length
DIM = 1024        # model dim
HD = 64           # head dim
NT = S // P       # 16 token tiles
N_CT = DIM // P   # 8 contraction tiles for qkv proj
QKV = 384         # per-core projection rows: 256 q + 64 k + 64 v
CW = 512          # tq chunk width
NCH = S // CW     # 4 chunks

_NC_CACHE = {}


def _build_kernel_program():
    import concourse.bass as bass
    import concourse.tile as tile
    from concourse import bacc, mybir

    dt = mybir.dt
    f32, bf16 = dt.float32, dt.bfloat16
    AF = mybir.ActivationFunctionType

    nc = bacc.Bacc("TRN2", target_bir_lowering=False, debug=False)

    # host-prepped layouts chosen for maximal DMA contiguity: xt has 8KB
    # contiguous per (partition, chunk), wqkv 6KB per partition, outp tiles
    # are fully contiguous 128KB blocks
    xt = nc.dram_tensor("xt", [P, NCH, N_CT, CW], bf16, kind="ExternalInput").ap()
    wqkv = nc.dram_tensor("wqkv", [P, N_CT, QKV], bf16, kind="ExternalInput").ap()
    wo = nc.dram_tensor("wo", [256, DIM], bf16, kind="ExternalInput").ap()
    cost = nc.dram_tensor("cost", [P, S], bf16, kind="ExternalInput").ap()
    sint = nc.dram_tensor("sint", [P, S], bf16, kind="ExternalInput").ap()
    rott = nc.dram_tensor("rott", [P, P], bf16, kind="ExternalInput").ap()
    ident = nc.dram_tensor("ident", [P, P], bf16, kind="ExternalInput").ap()
    ltri = nc.dram_tensor("ltri", [P, P], bf16, kind="ExternalInput").ap()
    ida = nc.dram_tensor("ida", [P, P], bf16, kind="ExternalInput").ap()
    outp = nc.dram_tensor("outp", [NT, 2, P, CW], bf16, kind="ExternalOutput").ap()

    with tile.TileContext(nc) as tc:
        with ExitStack() as ctx:
            _emit(ctx, tc, nc, mybir, bass, dict(
                xt=xt, wqkv=wqkv, wo=wo, cost=cost, sint=sint, rott=rott,
                ident=ident, ltri=ltri, ida=ida, outp=outp,
            ), f32, bf16, AF)
    nc.compile()
    return nc


def _emit(ctx, tc, nc, mybir, bass, io, f32, bf16, AF):
    tp = tc.tile_pool

    const = ctx.enter_context(tp(name="const", bufs=1))
    persist = ctx.enter_context(tp(name="persist", bufs=1))
    tmp = ctx.enter_context(tp(name="tmp", bufs=4))
    ptp = ctx.enter_context(tp(name="pt", bufs=6))
    # PSUM pools: total exactly 16KB/partition (8 banks)
    scp = ctx.enter_context(tp(name="sc", bufs=2, space="PSUM"))   # 2x 4KB
    pvp = ctx.enter_context(tp(name="pv", bufs=2, space="PSUM"))   # 2x 2KB
    fil = ctx.enter_context(tp(name="fil", bufs=2, space="PSUM"))  # 2x 2KB

    # ---- DMA in first-need order, split across BOTH issue queues ----
    # DMA_DIRECT2D costs ~600ns of serial issue time per descriptor, so the
    # startup-critical set is split between the Sync and Scalar (Activation)
    # HWDGE queues: weights+consts on scalar (idle until the first exp),
    # activations on sync.  Coarse bundles keep the issue count low; the
    # {ct0,ct1}/{ct2-7} split lets the first proj matmul start early.
    wqkv_sb = persist.tile([P, N_CT, QKV], bf16, name="wqkv_sb", tag="wqkv_sb")
    # xt_sb indexed [partition, chunk, ct, token-within-chunk]
    xt_sb = persist.tile([P, NCH, N_CT, CW], bf16, name="xt_sb", tag="xt_sb")
    cost_sb = persist.tile([P, S], bf16, name="cost_sb", tag="cost_sb")
    sint_sb = persist.tile([P, S], bf16, name="sint_sb", tag="sint_sb")
    rott_sb = const.tile([P, P], bf16, tag="rott")
    ida_sb = const.tile([P, P], bf16, tag="ida")
    ltri_sb = const.tile([P, P], bf16, tag="ltri")
    ident_sb = const.tile([P, P], bf16, tag="ident")
    # both HWDGE queues share the same 16 DMA engines, so GLOBAL issue order
    # is priority order; the n1 prefetch must come after everything critical
    nc.scalar.dma_start(ida_sb[:], io["ida"])          # warmup dummies need it
    nc.sync.dma_start(xt_sb[:, 0, 0:2, :], io["xt"][:, 0, 0:2, :])
    nc.scalar.dma_start(wqkv_sb[:, 0:4, :], io["wqkv"][:, 0:4, :])
    nc.sync.dma_start(xt_sb[:, 0, 2:N_CT, :], io["xt"][:, 0, 2:N_CT, :])
    nc.scalar.dma_start(wqkv_sb[:, 4:N_CT, :], io["wqkv"][:, 4:N_CT, :])
    nc.scalar.dma_start(rott_sb[:], io["rott"])
    nc.sync.dma_start(cost_sb[:, 0:CW], io["cost"][:, 0:CW])
    nc.sync.dma_start(sint_sb[:, 0:CW], io["sint"][:, 0:CW])
    nc.scalar.dma_start(ltri_sb[:], io["ltri"])
    nc.scalar.dma_start(ident_sb[:], io["ident"])
    # token chunk n1 after the n0-critical set so c0-era fillers can start
    # projecting n1 on time; n2/n3/wo are DMA'd later from filler tasks
    nc.sync.dma_start(xt_sb[:, 1, :, :], io["xt"][:, 1, :, :])
    nc.sync.dma_start(cost_sb[:, CW:2 * CW], io["cost"][:, CW:2 * CW])
    nc.sync.dma_start(sint_sb[:, CW:2 * CW], io["sint"][:, CW:2 * CW])
    wo_sb = persist.tile([P, 2, DIM], bf16, name="wo_sb", tag="wo_sb")

    def dma_chunk_task(n):
        def f():
            sl = slice(n * CW, (n + 1) * CW)
            nc.sync.dma_start(xt_sb[:, n, :, :], io["xt"][:, n, :, :])
            nc.sync.dma_start(cost_sb[:, sl], io["cost"][:, sl])
            nc.sync.dma_start(sint_sb[:, sl], io["sint"][:, sl])
        return [f]

    def dma_wo_task():
        def f():
            nc.sync.dma_start(wo_sb[:], io["wo"].rearrange("(a p) e -> p a e", p=P))
        return [f]

    # ---- persistent SBUF activations ----
    q01T = persist.tile([P, S], bf16, name="q01T", tag="q01T")
    q23T = persist.tile([P, S], bf16, name="q23T", tag="q23T")
    kvT = persist.tile([P, S], bf16, name="kvT", tag="kvT")
    q01r = persist.tile([P, S], bf16, name="q01r", tag="q01r")
    q23r = persist.tile([P, S], bf16, name="q23r", tag="q23r")
    krep = persist.tile([P, S], bf16, name="krep", tag="krep")
    # v_sb per tk tile: [ones | zeros(63) | v(64) | ones], 129 cols.
    # Head 0's PV stationary = cols 64:129 ([v|ones], M=65, out rows 0:65):
    # attn rows 0:64, denom row 64.  Head 1's = cols 0:128 ([ones|z63|v],
    # M=128, out rows 0:128): denom row 0, zeros rows 1:64, attn rows
    # 64:128.  Both heads' attn land partition-aligned with attnT and both
    # denoms sit on 32-aligned partitions, so normalization is just
    # reciprocal -> K=1 broadcast-matmul -> elementwise mul. No transposes.
    VW = P + 1
    v_sb = persist.tile([P, NT, VW + 1], bf16, name="v_sb", tag="v_sb")
    nc.vector.memset(v_sb[:, :, 1:HD], 0.0)
    nc.vector.memset(v_sb[:, :, 0:1], 1.0)
    nc.vector.memset(v_sb[:, :, P:VW], 1.0)
    onesc = const.tile([P, P], bf16, tag="onesc")
    nc.vector.memset(onesc[0:1, :], 1.0)
    nc.vector.memset(onesc[64:65, :], 1.0)
    attnT = [persist.tile([P, S], bf16, name="attnT01", tag="attnT01"),
             persist.tile([P, S], bf16, name="attnT23", tag="attnT23")]
    qrs = [q01r, q23r]

    # ---- micro-task fillers (each ~2 matmuls of PE work or less) ----
    qkv_dst = {"q01": (q01T, 0), "q23": (q23T, P), "kv": (kvT, 2 * P)}

    # every filler task's PSUM (fil-tag) lifetime is self-contained within
    # the task, so inline rb/po allocations between tasks never cross a
    # live accumulation in the 2-slot fil rotation
    def proj_tasks(dst_name, n):
        """two self-contained tasks of 4 ct-matmuls each; the second half
        accumulates into the destination with a DVE add so each task's PSUM
        lifetime stays within the task"""
        dst, mt = qkv_dst[dst_name]
        sl = slice(n * CW, (n + 1) * CW)

        def half(k):
            def f():
                ps = fil.tile([P, CW], f32, name="ps", tag="fil")
                for cti in range(4 * k, 4 * k + 4):
                    nc.tensor.matmul(
                        ps, wqkv_sb[:, cti, mt:mt + P], xt_sb[:, n, cti, :],
                        start=(cti == 4 * k), stop=(cti == 4 * k + 3),
                    )
                if k == 0:
                    nc.vector.tensor_copy(dst[:, sl], ps)
                else:
                    nc.vector.tensor_add(dst[:, sl], dst[:, sl], ps)
            return f
        return [half(0), half(1)]

    def rope_tasks(src, dst, rows, n):
        """one task: rot-matmul + cos-mul + sin-mul + add"""
        sl = slice(n * CW, (n + 1) * CW)

        def f():
            psr = fil.tile([P, CW], f32, name="psr", tag="fil")[:rows, :]
            nc.tensor.matmul(psr, rott_sb[:rows, :rows], src[:rows, sl],
                             start=True, stop=True)
            t1 = tmp.tile([P, CW], bf16, name="ropet1", tag="rope")[:rows]
            nc.gpsimd.tensor_mul(t1, src[:rows, sl], cost_sb[:rows, sl])
            t2 = tmp.tile([P, CW], bf16, name="ropet2", tag="rope")[:rows]
            nc.vector.tensor_mul(t2, psr, sint_sb[:rows, sl])
            nc.vector.tensor_add(dst[:rows, sl], t1, t2)
        return [f]

    def krep_task(n):
        # replicate roped k to partitions 64:128 via PE (col-group 64) + DVE
        # evict -- an SBUF->SBUF DMA here would queue behind megabytes of
        # input DMA and stall every score matmul
        def f():
            sl = slice(n * CW, (n + 1) * CW)
            pk = fil.tile([P, CW], f32, name="pk", tag="fil")
            nc.tensor.matmul(pk[64:128, :], ida_sb[0:64, 0:HD], krep[0:64, sl],
                             start=True, stop=True, tile_position=(0, 64))
            nc.vector.tensor_copy(krep[64:128, sl], pk[64:128, :])
        return [f]

    def v_task(j):
        def f():
            pst = fil.tile([P, CW], bf16, name="pst", tag="fil")[:, :HD]
            nc.tensor.transpose(pst, kvT[64:128, j * P:(j + 1) * P],
                                ident_sb[64:128, 0:HD])
            nc.vector.tensor_copy(v_sb[:, j, HD:P], pst)
        return [f]

    def wo_task(tt, e):
        def f():
            osb = tmp.tile([P, CW], bf16, name="osb", tag="osb", bufs=3)
            po = fil.tile([P, CW], f32, name="po", tag="fil")
            nc.tensor.matmul(po, attnT[0][:, tt * P:(tt + 1) * P],
                             wo_sb[:, 0, e * CW:(e + 1) * CW],
                             start=True, stop=False)
            nc.tensor.matmul(po, attnT[1][:, tt * P:(tt + 1) * P],
                             wo_sb[:, 1, e * CW:(e + 1) * CW],
                             start=False, stop=True)
            nc.vector.tensor_copy(osb[:], po)
            nc.sync.dma_start(io["outp"][tt, e, :, :], osb[:])
        return [f]

    def kv_chain(n):
        # kv proj + k-rope + krep for token chunk n (scores j>=4n need krep)
        return (proj_tasks("kv", n) + rope_tasks(kvT, krep, HD, n)
                + krep_task(n))

    def v_chain(n):
        t = []
        for jj in range(4 * n, 4 * n + 4):
            t += v_task(jj)
        return t

    # persistent rcp staging rows (64 for head0, 32 for head1), double-
    # buffered by tile parity: [partition, parity, head, col]
    rcp = persist.tile([P, 2, 2, P], bf16, name="rcp", tag="rcp")

    def norm_tile(hp, c, tt, pvE, pvO):
        """Normalize 128-col tile tt of chunk (hp,c) straight out of the pv
        PSUM into attnT: reciprocal of the denom rows, K=1 broadcast-matmul
        across partitions, elementwise mul. Runs inline in the j-loop right
        after PV(j=4c+tt); later PV matmuls don't touch these columns."""
        sl = slice(tt * P, (tt + 1) * P)
        g = c * CW + tt * P
        pr = tt % 2
        with nc.allow_low_precision(reason="bf16 softmax denom recip"):
            nc.vector.reciprocal(rcp[64:65, pr, 0, :], pvE[HD:HD + 1, sl])
            nc.vector.reciprocal(rcp[0:1, pr, 1, :], pvO[0:1, sl])
        rb = fil.tile([P, 2, P], f32, name="rb", tag="fil")
        nc.tensor.matmul(rb[:, 0, :], onesc[64:65, :], rcp[64:65, pr, 0, :],
                         start=True, stop=True)
        nc.tensor.matmul(rb[:, 1, :], onesc[0:1, :], rcp[0:1, pr, 1, :],
                         start=True, stop=True)
        # TensorTensor may read only one PSUM operand -> stage rb in SBUF
        rbs = tmp.tile([P, 2, P], bf16, name="rbs", tag="rbs", bufs=2)
        nc.vector.tensor_copy(rbs[:], rb)
        nc.vector.tensor_mul(attnT[hp][0:HD, g:g + P], pvE[0:HD, sl],
                             rbs[0:HD, 0, :])
        nc.vector.tensor_mul(attnT[hp][HD:P, g:g + P], pvO[HD:P, sl],
                             rbs[HD:P, 1, :])

    # schedule: (hp, c) -> (filler list, per-j budget). Every producer runs
    # at least one chunk before its consumer; kv/krep/v for chunk c+1 are
    # produced early inside chunk c+1 itself (consumed before j reaches 4c+4).
    # schedule: every producer runs at least one chunk before its consumer;
    # kv/krep/v for chunk c+1 are produced early inside chunk c+1 itself
    # (consumed before j reaches 4c+4)
    sched = {
        # (0,0): DMA-independent work first (q23 n0 uses resident xt n0) so
        # the in-order PE FIFO never stalls on the xt-n1 DMA during the
        # first chunk; q01-n1 tasks go last (run near chunk end, DMA landed)
        (0, 0): (proj_tasks("q23", 0) + rope_tasks(q23T, q23r, P, 0)
                 + dma_chunk_task(2) + proj_tasks("q01", 1)
                 + rope_tasks(q01T, q01r, P, 1), 2),
        (0, 1): (dma_chunk_task(3) + kv_chain(1) + v_chain(1)
                 + proj_tasks("q01", 2) + rope_tasks(q01T, q01r, P, 2)
                 + proj_tasks("q23", 1) + rope_tasks(q23T, q23r, P, 1), 2),
        (0, 2): (dma_wo_task() + kv_chain(2) + v_chain(2)
                 + proj_tasks("q01", 3) + rope_tasks(q01T, q01r, P, 3), 1),
        (0, 3): (kv_chain(3) + v_chain(3), 1),
        # q23 chains for the last two hp1 chunks trickle through hp1 at
        # budget 1: keeps hp1's per-j PE near the exp cadence (denser for
        # HAM) without delaying critical ops
        (1, 0): (proj_tasks("q23", 2), 1),
        (1, 1): (rope_tasks(q23T, q23r, P, 2)
                 + proj_tasks("q23", 3), 1),
        (1, 2): (rope_tasks(q23T, q23r, P, 3), 1),
        (1, 3): ([], 1),
    }

    # ---- preamble: tokens 0:512 projected + roped (critical path) ----
    # PE warmup during the DMA wait: dummy matmuls on already-landed tiles
    # keep the PE busy so HAM un-throttles before the real preamble, and the
    # FIFO drains right as the wqkv halves land.
    for _w in range(4):
        wps = scp.tile([P, 2, CW], f32, name="warm", tag="sc")
        nc.tensor.matmul(wps[:, 0, :], ida_sb, xt_sb[:, 0, 0, :],
                         start=True, stop=True)
    # kv proj then q01 proj back-to-back on the PE; kv's serial cross-engine
    # chain (evict -> rope -> krep) overlaps q01's matmuls, and q01 is not
    # stuck behind the krep matmul's semaphore wait in the in-order PE FIFO.
    kvc = kv_chain(0)
    q01c = proj_tasks("q01", 0) + rope_tasks(q01T, q01r, P, 0)
    for t in kvc[:5] + q01c[:5] + kvc[5:] + q01c[5:] + v_chain(0):
        t()

    def scores(hp, c, j):
        """S^T for both heads of pair hp, tk tile j, tq chunk c.
        Diagonal j also accumulates a -60000 strictly-lower-tri block so the
        later exp zeroes masked positions (PE-side masking)."""
        lo = max(0, j * P - c * CW)
        diag = j >= 4 * c
        sc = scp.tile([P, 2, CW], f32, name="sc", tag="sc")
        for h in range(2):
            nc.tensor.matmul(
                sc[:, h, lo:CW], krep[64 * h:64 * h + 64, j * P:(j + 1) * P],
                qrs[hp][64 * h:64 * h + 64, c * CW + lo:(c + 1) * CW],
                start=True, stop=not diag, tile_position=(64 * h, 0),
            )
        if diag:
            for h in range(2):
                nc.tensor.matmul(
                    sc[:, h, lo:lo + P], ida_sb, ltri_sb,
                    start=False, stop=True, skip_group_check=True,
                )
        return sc, lo

    first = True
    sc_cur = lo_cur = None
    deferred = []
    for hp in range(2):
        for c in range(NCH):
            jmax = 4 * c + 3
            base_fillers, budget = sched[(hp, c)]
            fillers = deferred + base_fillers
            deferred = []
            fi = 0
            if first:
                sc_cur, lo_cur = scores(0, 0, 0)
                first = False
            pvE = pvp.tile([P, CW], f32, name="pvE", tag="pv")
            pvO = pvp.tile([P, CW], f32, name="pvO", tag="pv")
            for j in range(jmax + 1):
                sc, lo = sc_cur, lo_cur
                pt = ptp.tile([P, 2, CW], bf16, name="pt", tag="pt")
                nc.scalar.activation(pt[:, :, lo:CW], sc[:, :, lo:CW],
                                     AF.Exp, scale=0.125)
                # scores(j+1) IMMEDIATELY after exp(j) in the PE FIFO so the
                # ACT engine is never starved behind filler lumps (safe
                # intra-chunk: all its inputs were produced >=1 j earlier)
                if j < jmax:
                    sc_cur, lo_cur = scores(hp, c, j + 1)
                for _ in range(budget):
                    if fi < len(fillers):
                        fillers[fi]()
                        fi += 1
                st, sp = (j == 0), (j == jmax)
                nc.tensor.matmul(pvE[0:HD + 1, lo:CW], v_sb[:, j, HD:VW],
                                 pt[:, 0, lo:CW], start=st, stop=sp)
                nc.tensor.matmul(pvO[0:P, lo:CW], v_sb[:, j, 0:P],
                                 pt[:, 1, lo:CW], start=st, stop=sp)
                # norm for tile tt is emitted one j AFTER its last PV so the
                # PE never waits on the just-issued DVE reciprocal; wo lags
                # one more j so it never waits on the attnT muls
                if j - 1 >= 4 * c:
                    norm_tile(hp, c, j - 1 - 4 * c, pvE, pvO)
                if hp == 1 and j - 2 >= 4 * c:
                    for t in wo_task(4 * c + j - 2 - 4 * c, 0) + \
                             wo_task(4 * c + j - 2 - 4 * c, 1):
                        t()
            while fi < len(fillers):
                fillers[fi]()
                fi += 1
            # next chunk's first scores: after the filler drain (its inputs
            # may be produced by this chunk's last fillers)
            if (hp, c) != (1, NCH - 1):
                nhp, ncc = (hp, c + 1) if c < NCH - 1 else (hp + 1, 0)
                sc_cur, lo_cur = scores(nhp, ncc, 0)
            norm_tile(hp, c, 3, pvE, pvO)
            if hp == 1:
                # tile 2's wo runs now; tile 3's rides into the next chunk
                for t in wo_task(4 * c + 2, 0) + wo_task(4 * c + 2, 1):
                    t()
                deferred += wo_task(4 * c + 3, 0) + wo_task(4 * c + 3, 1)

    # ---- tail: only the final tile's wo remains ----
    for t in deferred:
        t()


def _host_inputs(X, cos, sin, Wq, Wk, Wv, Wo):
    """Build the 8 per-core input maps (host-side sharding + layout prep)."""
    cosT = np.ascontiguousarray(cos.T)  # [64, 2048]
    sinT = np.ascontiguousarray(sin.T)
    cost = np.concatenate([cosT, cosT], 0).astype(BF16)  # [128, 2048]
    sint = np.concatenate([sinT, sinT], 0).astype(BF16)
    rott = np.zeros((P, P), np.float32)
    idx = np.arange(0, P, 2)
    rott[idx, idx + 1] = 1.0    # RT[2i, 2i+1] = +1
    rott[idx + 1, idx] = -1.0   # RT[2i+1, 2i] = -1
    rott = rott.astype(BF16)
    ident = np.zeros((P, P), np.float32)
    ident[0:64, 0:64] = np.eye(64)
    ident[64:128, 0:64] = np.eye(64)   # same I64 available at base partition 64
    ident = ident.astype(BF16)
    # strictly-lower-triangular -60000: added into scores before exp so the
    # upper-left (tk > tq) of each diagonal block becomes exp(-inf) = 0
    ltri = np.tril(np.full((P, P), -60000.0, np.float32), k=-1).astype(BF16)
    ida = np.eye(P, dtype=np.float32).astype(BF16)

    # xt host layout [p, chunk, ct, t]: 8KB contiguous per (p, chunk) DMA run
    xts = [
        np.ascontiguousarray(
            X[b].T.astype(BF16).reshape(8, P, 4, CW).transpose(1, 2, 0, 3))
        for b in range(X.shape[0])
    ]

    in_maps = []
    for c in range(8):
        b, g = c // 4, c % 4
        wqkv = np.concatenate(
            [Wq[256 * g:256 * (g + 1)], Wk[64 * g:64 * (g + 1)], Wv[64 * g:64 * (g + 1)]], 0
        ).T.astype(BF16)                                   # [1024, 384]
        # wqkv host layout [p, ct, d]: 6KB contiguous per partition
        wqkv = np.ascontiguousarray(wqkv.reshape(8, P, QKV).transpose(1, 0, 2))
        wog = np.ascontiguousarray(Wo[:, 256 * g:256 * (g + 1)].T).astype(BF16)  # [256, 1024]
        in_maps.append({
            "xt": xts[b], "wqkv": wqkv, "wo": wog,
            "cost": cost, "sint": sint, "rott": rott, "ident": ident,
            "ltri": ltri, "ida": ida,
        })
    return in_maps


def get_nc():
    if "nc" not in _NC_CACHE:
        _NC_CACHE["nc"] = _build_kernel_program()
    return _NC_CACHE["nc"]


def _install_ntff_hook():
    """The agent image's antenv lacks axon_hooks; recreate it so trace=True
    can reach the terminal's NRT profiler (timing only, not needed for
    correctness)."""
    import types
    if "antenv.axon_hooks" in sys.modules:
        return
    try:
        import antenv
        m = types.ModuleType("antenv.axon_hooks")
        holder = {"v": None}
        m.set_axon_ntff_profile_hook = lambda h: holder.__setitem__("v", h)
        m.get_axon_ntff_profile_hook = lambda: holder["v"]
        sys.modules["antenv.axon_hooks"] = m
        antenv.axon_hooks = m
        from trn_agent_boot.trn_boot import _ntff_profile_via_ctypes
        m.set_axon_ntff_profile_hook(
            _ntff_profile_via_ctypes("/opt/axon/libaxon_pjrt.so"))
    except Exception:
        pass


def kernel(X, freqs_cos, freqs_sin, Wq, Wk, Wv, Wo, _trace=False):
    from concourse.bass_utils import run_bass_kernel_spmd

    if _trace:
        _install_ntff_hook()

    X = np.asarray(X, np.float32)
    in_maps = _host_inputs(
        X, np.asarray(freqs_cos, np.float32), np.asarray(freqs_sin, np.float32),
        np.asarray(Wq, np.float32), np.asarray(Wk, np.float32),
        np.asarray(Wv, np.float32), np.asarray(Wo, np.float32),
    )
    nc = get_nc()
    res = run_bass_kernel_spmd(nc, in_maps, core_ids=list(range(8)), trace=_trace)
    out = np.zeros((2, S, DIM), np.float32)
    for c in range(8):
        o = res.results[c]["outp"].astype(np.float32)   # [16, 2, 128, 512]
        out[c // 4] += o.transpose(0, 2, 1, 3).reshape(S, DIM)
    if _trace:
        kernel.last_result = res
    return out



# revision 45
# speedup vs baseline: 1.0279x; 1.0279x over previous
"""Trainium2 Bass kernel for GQA attention (b=2, s=2048, dim=1024, 16 q / 4 kv heads).

Sharding: 8 cores = 2 (batch) x 4 (head groups). Each core owns one batch
element and 4 q-heads + 1 kv-head (Wq/Wk/Wv column-sharded, Wo row-sharded).
Host pre-transposes everything contraction-major in bf16; host sums the 4
Wo partials per batch element in fp32.

v3 structure:
  - loops: head-pair hp (outer) -> tq chunk c of 512 -> tk tile j.
  - scores for both heads of a pair in ONE [128, 2, 512] fp32 PSUM tile
    (row-tiled concurrent matmuls at tile_position (0,0)/(64,0)); exp is a
    single ScalarE instruction per (hp, c, j); ScalarE runs ONLY exp.
  - causal diag mask applied ON THE PE: an extra accumulate-matmul adds a
    strictly-lower-triangular -60000 constant (ida stationary, ltri moving)
    into the scores PSUM before exp -> no cross-engine hop in the chain.
  - per-j emission order: exp(j) | fillers | scores(j+1) | PV(j), so the PE
    FIFO always has filler work while exp(j) runs and PV(j) never blocks
    the next scores. scores(c+1,0) is emitted before completion(c).
  - fillers are micro-tasks (2 matmuls each) scheduled so every input is
    produced a chunk ahead of first use and only DMA-landed data is touched.
  - normalization transpose path entirely in bf16 (pvs/pT/rec4/anrm).
"""

import sys
from contextlib import ExitStack

for _p in ("/opt/trn_rl_repo",):
    if _p not in sys.path:
        sys.path.insert(0, _p)

import numpy as np
import ml_dtypes

BF16 = ml_dtypes.bfloat16

P = 128
S = 2048          # sequence length
DIM = 1024        # model dim
HD = 64           # head dim
NT = S // P       # 16 token tiles
N_CT = DIM // P   # 8 contraction tiles for qkv proj
QKV = 384         # per-core projection rows: 256 q + 64 k + 64 v
CW = 512          # tq chunk width
NCH = S // CW     # 4 chunks

_NC_CACHE = {}


def _build_kernel_program():
    import concourse.bass as bass
    import concourse.tile as tile
    from concourse import bacc, mybir

    dt = mybir.dt
    f32, bf16 = dt.float32, dt.bfloat16
    AF = mybir.ActivationFunctionType

    nc = bacc.Bacc("TRN2", target_bir_lowering=False, debug=False)

    # host-prepped layouts chosen for maximal DMA contiguity: xt has 8KB
    # contiguous per (partition, chunk), wqkv 6KB per partition, outp tiles
    # are fully contiguous 128KB blocks
    xt = nc.dram_tensor("xt", [P, NCH, N_CT, CW], bf16, kind="ExternalInput").ap()
    wqkv = nc.dram_tensor("wqkv", [P, N_CT, QKV], bf16, kind="ExternalInput").ap()
    wo = nc.dram_tensor("wo", [256, DIM], bf16, kind="ExternalInput").ap()
    cost = nc.dram_tensor("cost", [P, S], bf16, kind="ExternalInput").ap()
    sint = nc.dram_tensor("sint", [P, S], bf16, kind="ExternalInput").ap()
    rott = nc.dram_tensor("rott", [P, P], bf16, kind="ExternalInput").ap()
    ident = nc.dram_tensor("ident", [P, P], bf16, kind="ExternalInput").ap()
    ltri = nc.dram_tensor("ltri", [P, P], bf16, kind="ExternalInput").ap()
    ida = nc.dram_tensor("ida", [P, P], bf16, kind="ExternalInput").ap()
    outp = nc.dram_tensor("outp", [NT, 2, P, CW], bf16, kind="ExternalOutput").ap()

    with tile.TileContext(nc) as tc:
        with ExitStack() as ctx:
            _emit(ctx, tc, nc, mybir, bass, dict(
                xt=xt, wqkv=wqkv, wo=wo, cost=cost, sint=sint, rott=rott,
                ident=ident, ltri=ltri, ida=ida, outp=outp,
            ), f32, bf16, AF)
    nc.compile()
    return nc


def _emit(ctx, tc, nc, mybir, bass, io, f32, bf16, AF):
    tp = tc.tile_pool

    const = ctx.enter_context(tp(name="const", bufs=1))
    persist = ctx.enter_context(tp(name="persist", bufs=1))
    tmp = ctx.enter_context(tp(name="tmp", bufs=4))
    ptp = ctx.enter_context(tp(name="pt", bufs=6))
    # PSUM pools: total exactly 16KB/partition (8 banks)
    scp = ctx.enter_context(tp(name="sc", bufs=2, space="PSUM"))   # 2x 4KB
    pvp = ctx.enter_context(tp(name="pv", bufs=2, space="PSUM"))   # 2x 2KB
    fil = ctx.enter_context(tp(name="fil", bufs=2, space="PSUM"))  # 2x 2KB

    # ---- DMA in first-need order, split across BOTH issue queues ----
    # DMA_DIRECT2D costs ~600ns of serial issue time per descriptor, so the
    # startup-critical set is split between the Sync and Scalar (Activation)
    # HWDGE queues: weights+consts on scalar (idle until the first exp),
    # activations on sync.  Coarse bundles keep the issue count low; the
    # {ct0,ct1}/{ct2-7} split lets the first proj matmul start early.
    wqkv_sb = persist.tile([P, N_CT, QKV], bf16, name="wqkv_sb", tag="wqkv_sb")
    # xt_sb indexed [partition, chunk, ct, token-within-chunk]
    xt_sb = persist.tile([P, NCH, N_CT, CW], bf16, name="xt_sb", tag="xt_sb")
    cost_sb = persist.tile([P, S], bf16, name="cost_sb", tag="cost_sb")
    sint_sb = persist.tile([P, S], bf16, name="sint_sb", tag="sint_sb")
    rott_sb = const.tile([P, P], bf16, tag="rott")
    ida_sb = const.tile([P, P], bf16, tag="ida")
    ltri_sb = const.tile([P, P], bf16, tag="ltri")
    ident_sb = const.tile([P, P], bf16, tag="ident")
    # both HWDGE queues share the same 16 DMA engines, so GLOBAL issue order
    # is priority order; the n1 prefetch must come after everything critical
    nc.scalar.dma_start(ida_sb[:], io["ida"])          # warmup dummies need it
    nc.sync.dma_start(xt_sb[:, 0, 0:2, :], io["xt"][:, 0, 0:2, :])
    nc.scalar.dma_start(wqkv_sb[:, 0:4, :], io["wqkv"][:, 0:4, :])
    nc.sync.dma_start(xt_sb[:, 0, 2:N_CT, :], io["xt"][:, 0, 2:N_CT, :])
    nc.scalar.dma_start(wqkv_sb[:, 4:N_CT, :], io["wqkv"][:, 4:N_CT, :])
    nc.scalar.dma_start(rott_sb[:], io["rott"])
    nc.sync.dma_start(cost_sb[:, 0:CW], io["cost"][:, 0:CW])
    nc.sync.dma_start(sint_sb[:, 0:CW], io["sint"][:, 0:CW])
    nc.scalar.dma_start(ltri_sb[:], io["ltri"])
    nc.scalar.dma_start(ident_sb[:], io["ident"])
    # token chunk n1 after the n0-critical set so c0-era fillers can start
    # projecting n1 on time; n2/n3/wo are DMA'd later from filler tasks
    nc.sync.dma_start(xt_sb[:, 1, :, :], io["xt"][:, 1, :, :])
    nc.sync.dma_start(cost_sb[:, CW:2 * CW], io["cost"][:, CW:2 * CW])
    nc.sync.dma_start(sint_sb[:, CW:2 * CW], io["sint"][:, CW:2 * CW])
    wo_sb = persist.tile([P, 2, DIM], bf16, name="wo_sb", tag="wo_sb")

    def dma_chunk_task(n):
        def f():
            sl = slice(n * CW, (n + 1) * CW)
            nc.sync.dma_start(xt_sb[:, n, :, :], io["xt"][:, n, :, :])
            nc.sync.dma_start(cost_sb[:, sl], io["cost"][:, sl])
            nc.sync.dma_start(sint_sb[:, sl], io["sint"][:, sl])
        return [f]

    def dma_wo_task():
        def f():
            nc.sync.dma_start(wo_sb[:], io["wo"].rearrange("(a p) e -> p a e", p=P))
        return [f]

    # ---- persistent SBUF activations ----
    q01T = persist.tile([P, S], bf16, name="q01T", tag="q01T")
    q23T = persist.tile([P, S], bf16, name="q23T", tag="q23T")
    kvT = persist.tile([P, S], bf16, name="kvT", tag="kvT")
    q01r = persist.tile([P, S], bf16, name="q01r", tag="q01r")
    q23r = persist.tile([P, S], bf16, name="q23r", tag="q23r")
    krep = persist.tile([P, S], bf16, name="krep", tag="krep")
    # v_sb per tk tile: [ones | zeros(63) | v(64) | ones], 129 cols.
    # Head 0's PV stationary = cols 64:129 ([v|ones], M=65, out rows 0:65):
    # attn rows 0:64, denom row 64.  Head 1's = cols 0:128 ([ones|z63|v],
    # M=128, out rows 0:128): denom row 0, zeros rows 1:64, attn rows
    # 64:128.  Both heads' attn land partition-aligned with attnT and both
    # denoms sit on 32-aligned partitions, so normalization is just
    # reciprocal -> K=1 broadcast-matmul -> elementwise mul. No transposes.
    VW = P + 1
    v_sb = persist.tile([P, NT, VW + 1], bf16, name="v_sb", tag="v_sb")
    nc.vector.memset(v_sb[:, :, 1:HD], 0.0)
    nc.vector.memset(v_sb[:, :, 0:1], 1.0)
    nc.vector.memset(v_sb[:, :, P:VW], 1.0)
    onesc = const.tile([P, P], bf16, tag="onesc")
    nc.vector.memset(onesc[0:1, :], 1.0)
    nc.vector.memset(onesc[64:65, :], 1.0)
    attnT = [persist.tile([P, S], bf16, name="attnT01", tag="attnT01"),
             persist.tile([P, S], bf16, name="attnT23", tag="attnT23")]
    qrs = [q01r, q23r]

    # ---- micro-task fillers (each ~2 matmuls of PE work or less) ----
    qkv_dst = {"q01": (q01T, 0), "q23": (q23T, P), "kv": (kvT, 2 * P)}

    # every filler task's PSUM (fil-tag) lifetime is self-contained within
    # the task, so inline rb/po allocations between tasks never cross a
    # live accumulation in the 2-slot fil rotation
    def proj_tasks(dst_name, n):
        """two self-contained tasks of 4 ct-matmuls each; the second half
        accumulates into the destination with a DVE add so each task's PSUM
        lifetime stays within the task"""
        dst, mt = qkv_dst[dst_name]
        sl = slice(n * CW, (n + 1) * CW)

        def half(k):
            def f():
                ps = fil.tile([P, CW], f32, name="ps", tag="fil")
                for cti in range(4 * k, 4 * k + 4):
                    nc.tensor.matmul(
                        ps, wqkv_sb[:, cti, mt:mt + P], xt_sb[:, n, cti, :],
                        start=(cti == 4 * k), stop=(cti == 4 * k + 3),
                    )
                if k == 0:
                    nc.vector.tensor_copy(dst[:, sl], ps)
                else:
                    nc.vector.tensor_add(dst[:, sl], dst[:, sl], ps)
            return f
        return [half(0), half(1)]

    def rope_tasks(src, dst, rows, n):
        """one task: rot-matmul + cos-mul + sin-mul + add"""
        sl = slice(n * CW, (n + 1) * CW)

        def f():
            psr = fil.tile([P, CW], f32, name="psr", tag="fil")[:rows, :]
            nc.tensor.matmul(psr, rott_sb[:rows, :rows], src[:rows, sl],
                             start=True, stop=True)
            t1 = tmp.tile([P, CW], bf16, name="ropet1", tag="rope")[:rows]
            nc.gpsimd.tensor_mul(t1, src[:rows, sl], cost_sb[:rows, sl])
            t2 = tmp.tile([P, CW], bf16, name="ropet2", tag="rope")[:rows]
            nc.vector.tensor_mul(t2, psr, sint_sb[:rows, sl])
            nc.vector.tensor_add(dst[:rows, sl], t1, t2)
        return [f]

    def krep_task(n):
        # replicate roped k to partitions 64:128 via PE (col-group 64) + DVE
        # evict -- an SBUF->SBUF DMA here would queue behind megabytes of
        # input DMA and stall every score matmul
        def f():
            sl = slice(n * CW, (n + 1) * CW)
            pk = fil.tile([P, CW], f32, name="pk", tag="fil")
            nc.tensor.matmul(pk[64:128, :], ida_sb[0:64, 0:HD], krep[0:64, sl],
                             start=True, stop=True, tile_position=(0, 64))
            nc.vector.tensor_copy(krep[64:128, sl], pk[64:128, :])
        return [f]

    def v_task(j):
        def f():
            pst = fil.tile([P, CW], bf16, name="pst", tag="fil")[:, :HD]
            nc.tensor.transpose(pst, kvT[64:128, j * P:(j + 1) * P],
                                ident_sb[64:128, 0:HD])
            nc.vector.tensor_copy(v_sb[:, j, HD:P], pst)
        return [f]

    def wo_task(tt, e):
        def f():
            osb = tmp.tile([P, CW], bf16, name="osb", tag="osb", bufs=3)
            po = fil.tile([P, CW], f32, name="po", tag="fil")
            nc.tensor.matmul(po, attnT[0][:, tt * P:(tt + 1) * P],
                             wo_sb[:, 0, e * CW:(e + 1) * CW],
                             start=True, stop=False)
            nc.tensor.matmul(po, attnT[1][:, tt * P:(tt + 1) * P],
                             wo_sb[:, 1, e * CW:(e + 1) * CW],
                             start=False, stop=True)
            nc.vector.tensor_copy(osb[:], po)
            nc.sync.dma_start(io["outp"][tt, e, :, :], osb[:])
        return [f]

    def kv_chain(n):
        # kv proj + k-rope + krep for token chunk n (scores j>=4n need krep)
        return (proj_tasks("kv", n) + rope_tasks(kvT, krep, HD, n)
                + krep_task(n))

    def v_chain(n):
        t = []
        for jj in range(4 * n, 4 * n + 4):
            t += v_task(jj)
        return t

    # persistent rcp staging rows (64 for head0, 32 for head1), double-
    # buffered by tile parity: [partition, parity, head, col]
    rcp = persist.tile([P, 2, 2, P], bf16, name="rcp", tag="rcp")

    def norm_tile(hp, c, tt, pvE, pvO):
        """Normalize 128-col tile tt of chunk (hp,c) straight out of the pv
        PSUM into attnT: reciprocal of the denom rows, K=1 broadcast-matmul
        across partitions, elementwise mul. Runs inline in the j-loop right
        after PV(j=4c+tt); later PV matmuls don't touch these columns."""
        sl = slice(tt * P, (tt + 1) * P)
        g = c * CW + tt * P
        pr = tt % 2
        with nc.allow_low_precision(reason="bf16 softmax denom recip"):
            nc.vector.reciprocal(rcp[64:65, pr, 0, :], pvE[HD:HD + 1, sl])
            nc.vector.reciprocal(rcp[0:1, pr, 1, :], pvO[0:1, sl])
        rb = fil.tile([P, 2, P], f32, name="rb", tag="fil")
        nc.tensor.matmul(rb[:, 0, :], onesc[64:65, :], rcp[64:65, pr, 0, :],
                         start=True, stop=True)
        nc.tensor.matmul(rb[:, 1, :], onesc[0:1, :], rcp[0:1, pr, 1, :],
                         start=True, stop=True)
        # TensorTensor may read only one PSUM operand -> stage rb in SBUF
        rbs = tmp.tile([P, 2, P], bf16, name="rbs", tag="rbs", bufs=2)
        nc.vector.tensor_copy(rbs[:], rb)
        nc.vector.tensor_mul(attnT[hp][0:HD, g:g + P], pvE[0:HD, sl],
                             rbs[0:HD, 0, :])
        nc.vector.tensor_mul(attnT[hp][HD:P, g:g + P], pvO[HD:P, sl],
                             rbs[HD:P, 1, :])

    # schedule: (hp, c) -> (filler list, per-j budget). Every producer runs
    # at least one chunk before its consumer; kv/krep/v for chunk c+1 are
    # produced early inside chunk c+1 itself (consumed before j reaches 4c+4).
    # schedule: every producer runs at least one chunk before its consumer;
    # kv/krep/v for chunk c+1 are produced early inside chunk c+1 itself
    # (consumed before j reaches 4c+4)
    sched = {
        # (0,0): DMA-independent work first (q23 n0 uses resident xt n0) so
        # the in-order PE FIFO never stalls on the xt-n1 DMA during the
        # first chunk; q01-n1 tasks go last (run near chunk end, DMA landed)
        # per chunk: next-chunk-critical work (q proj+rope, kv chain) drains
        # FIRST so the next chunk's scores never wait on a fresh rope chain
        # at the transition; v tasks land just before their first PV use
        (0, 0): (dma_chunk_task(2) + proj_tasks("q01", 1)
                 + rope_tasks(q01T, q01r, P, 1)
                 + proj_tasks("q23", 0) + rope_tasks(q23T, q23r, P, 0), 2),
        (0, 1): (dma_chunk_task(3) + kv_chain(1) + v_chain(1)
                 + proj_tasks("q01", 2) + rope_tasks(q01T, q01r, P, 2)
                 + proj_tasks("q23", 1) + rope_tasks(q23T, q23r, P, 1), 2),
        (0, 2): (dma_wo_task() + kv_chain(2)
                 + v_chain(2)[:2]
                 + proj_tasks("q01", 3) + rope_tasks(q01T, q01r, P, 3)
                 + v_chain(2)[2:], 1),
        (0, 3): (kv_chain(3) + v_chain(3), 1),
        # q23 chains for the last two hp1 chunks trickle through hp1 at
        # budget 1: keeps hp1's per-j PE near the exp cadence (denser for
        # HAM) without delaying critical ops
        (1, 0): (proj_tasks("q23", 2), 1),
        (1, 1): (rope_tasks(q23T, q23r, P, 2)
                 + proj_tasks("q23", 3), 1),
        (1, 2): (rope_tasks(q23T, q23r, P, 3), 1),
        (1, 3): ([], 1),
    }

    # ---- preamble: tokens 0:512 projected + roped (critical path) ----
    # PE warmup during the DMA wait: dummy matmuls on already-landed tiles
    # keep the PE busy so HAM un-throttles before the real preamble, and the
    # FIFO drains right as the wqkv halves land.
    for _w in range(4):
        wps = scp.tile([P, 2, CW], f32, name="warm", tag="sc")
        nc.tensor.matmul(wps[:, 0, :], ida_sb, xt_sb[:, 0, 0, :],
                         start=True, stop=True)
    # kv proj then q01 proj back-to-back on the PE; kv's serial cross-engine
    # chain (evict -> rope -> krep) overlaps q01's matmuls, and q01 is not
    # stuck behind the krep matmul's semaphore wait in the in-order PE FIFO.
    kvc = kv_chain(0)
    q01c = proj_tasks("q01", 0) + rope_tasks(q01T, q01r, P, 0)
    for t in kvc[:5] + q01c[:5] + kvc[5:] + q01c[5:] + v_chain(0):
        t()

    def scores(hp, c, j):
        """S^T for both heads of pair hp, tk tile j, tq chunk c.
        Diagonal j also accumulates a -60000 strictly-lower-tri block so the
        later exp zeroes masked positions (PE-side masking)."""
        lo = max(0, j * P - c * CW)
        diag = j >= 4 * c
        sc = scp.tile([P, 2, CW], f32, name="sc", tag="sc")
        for h in range(2):
            nc.tensor.matmul(
                sc[:, h, lo:CW], krep[64 * h:64 * h + 64, j * P:(j + 1) * P],
                qrs[hp][64 * h:64 * h + 64, c * CW + lo:(c + 1) * CW],
                start=True, stop=not diag, tile_position=(64 * h, 0),
            )
        if diag:
            for h in range(2):
                nc.tensor.matmul(
                    sc[:, h, lo:lo + P], ida_sb, ltri_sb,
                    start=False, stop=True, skip_group_check=True,
                )
        return sc, lo

    first = True
    sc_cur = lo_cur = None
    deferred = []
    for hp in range(2):
        for c in range(NCH):
            jmax = 4 * c + 3
            base_fillers, budget = sched[(hp, c)]
            fillers = deferred + base_fillers
            deferred = []
            fi = 0
            if first:
                sc_cur, lo_cur = scores(0, 0, 0)
                first = False
            pvE = pvp.tile([P, CW], f32, name="pvE", tag="pv")
            pvO = pvp.tile([P, CW], f32, name="pvO", tag="pv")
            for j in range(jmax + 1):
                sc, lo = sc_cur, lo_cur
                pt = ptp.tile([P, 2, CW], bf16, name="pt", tag="pt")
                nc.scalar.activation(pt[:, :, lo:CW], sc[:, :, lo:CW],
                                     AF.Exp, scale=0.125)
                # scores(j+1) IMMEDIATELY after exp(j) in the PE FIFO so the
                # ACT engine is never starved behind filler lumps (safe
                # intra-chunk: all its inputs were produced >=1 j earlier)
                if j < jmax:
                    sc_cur, lo_cur = scores(hp, c, j + 1)
                for _ in range(budget):
                    if fi < len(fillers):
                        fillers[fi]()
                        fi += 1
                st, sp = (j == 0), (j == jmax)
                nc.tensor.matmul(pvE[0:HD + 1, lo:CW], v_sb[:, j, HD:VW],
                                 pt[:, 0, lo:CW], start=st, stop=sp)
                nc.tensor.matmul(pvO[0:P, lo:CW], v_sb[:, j, 0:P],
                                 pt[:, 1, lo:CW], start=st, stop=sp)
                # norm for tile tt is emitted one j AFTER its last PV so the
                # PE never waits on the just-issued DVE reciprocal; wo lags
                # one more j so it never waits on the attnT muls
                if j - 1 >= 4 * c:
                    norm_tile(hp, c, j - 1 - 4 * c, pvE, pvO)
                if hp == 1 and j - 2 >= 4 * c:
                    for t in wo_task(4 * c + j - 2 - 4 * c, 0) + \
                             wo_task(4 * c + j - 2 - 4 * c, 1):
                        t()
            while fi < len(fillers):
                fillers[fi]()
                fi += 1
            # next chunk's first scores: after the filler drain (its inputs
            # may be produced by this chunk's last fillers)
            if (hp, c) != (1, NCH - 1):
                nhp, ncc = (hp, c + 1) if c < NCH - 1 else (hp + 1, 0)
                sc_cur, lo_cur = scores(nhp, ncc, 0)
            norm_tile(hp, c, 3, pvE, pvO)
            if hp == 1:
                # tile 2's wo runs now; tile 3's rides into the next chunk
                for t in wo_task(4 * c + 2, 0) + wo_task(4 * c + 2, 1):
                    t()
                deferred += wo_task(4 * c + 3, 0) + wo_task(4 * c + 3, 1)

    # ---- tail: only the final tile's wo remains ----
    for t in deferred:
        t()


def _host_inputs(X, cos, sin, Wq, Wk, Wv, Wo):
    """Build the 8 per-core input maps (host-side sharding + layout prep)."""
    cosT = np.ascontiguousarray(cos.T)  # [64, 2048]
    sinT = np.ascontiguousarray(sin.T)
    cost = np.concatenate([cosT, cosT], 0).astype(BF16)  # [128, 2048]
    sint = np.concatenate([sinT, sinT], 0).astype(BF16)
    rott = np.zeros((P, P), np.float32)
    idx = np.arange(0, P, 2)
    rott[idx, idx + 1] = 1.0    # RT[2i, 2i+1] = +1
    rott[idx + 1, idx] = -1.0   # RT[2i+1, 2i] = -1
    rott = rott.astype(BF16)
    ident = np.zeros((P, P), np.float32)
    ident[0:64, 0:64] = np.eye(64)
    ident[64:128, 0:64] = np.eye(64)   # same I64 available at base partition 64
    ident = ident.astype(BF16)
    # strictly-lower-triangular -60000: added into scores before exp so the
    # upper-left (tk > tq) of each diagonal block becomes exp(-inf) = 0
    ltri = np.tril(np.full((P, P), -60000.0, np.float32), k=-1).astype(BF16)
    ida = np.eye(P, dtype=np.float32).astype(BF16)

    # xt host layout [p, chunk, ct, t]: 8KB contiguous per (p, chunk) DMA run
    xts = [
        np.ascontiguousarray(
            X[b].T.astype(BF16).reshape(8, P, 4, CW).transpose(1, 2, 0, 3))
        for b in range(X.shape[0])
    ]

    in_maps = []
    for c in range(8):
        b, g = c // 4, c % 4
        wqkv = np.concatenate(
            [Wq[256 * g:256 * (g + 1)], Wk[64 * g:64 * (g + 1)], Wv[64 * g:64 * (g + 1)]], 0
        ).T.astype(BF16)                                   # [1024, 384]
        # wqkv host layout [p, ct, d]: 6KB contiguous per partition
        wqkv = np.ascontiguousarray(wqkv.reshape(8, P, QKV).transpose(1, 0, 2))
        wog = np.ascontiguousarray(Wo[:, 256 * g:256 * (g + 1)].T).astype(BF16)  # [256, 1024]
        in_maps.append({
            "xt": xts[b], "wqkv": wqkv, "wo": wog,
            "cost": cost, "sint": sint, "rott": rott, "ident": ident,
            "ltri": ltri, "ida": ida,
        })
    return in_maps


def get_nc():
    if "nc" not in _NC_CACHE:
        _NC_CACHE["nc"] = _build_kernel_program()
    return _NC_CACHE["nc"]


def _install_ntff_hook():
    """The agent image's antenv lacks axon_hooks; recreate it so trace=True
    can reach the terminal's NRT profiler (timing only, not needed for
    correctness)."""
    import types
    if "antenv.axon_hooks" in sys.modules:
        return
    try:
        import antenv
        m = types.ModuleType("antenv.axon_hooks")
        holder = {"v": None}
        m.set_axon_ntff_profile_hook = lambda h: holder.__setitem__("v", h)
        m.get_axon_ntff_profile_hook = lambda: holder["v"]
        sys.modules["antenv.axon_hooks"] = m
        antenv.axon_hooks = m
        from trn_agent_boot.trn_boot import _ntff_profile_via_ctypes
        m.set_axon_ntff_profile_hook(
            _ntff_profile_via_ctypes("/opt/axon/libaxon_pjrt.so"))
    except Exception:
        pass


def kernel(X, freqs_cos, freqs_sin, Wq, Wk, Wv, Wo, _trace=False):
    from concourse.bass_utils import run_bass_kernel_spmd

    if _trace:
        _install_ntff_hook()

    X = np.asarray(X, np.float32)
    in_maps = _host_inputs(
        X, np.asarray(freqs_cos, np.float32), np.asarray(freqs_sin, np.float32),
        np.asarray(Wq, np.float32), np.asarray(Wk, np.float32),
        np.asarray(Wv, np.float32), np.asarray(Wo, np.float32),
    )
    nc = get_nc()
    res = run_bass_kernel_spmd(nc, in_maps, core_ids=list(range(8)), trace=_trace)
    out = np.zeros((2, S, DIM), np.float32)
    for c in range(8):
        o = res.results[c]["outp"].astype(np.float32)   # [16, 2, 128, 512]
        out[c // 4] += o.transpose(0, 2, 1, 3).reshape(S, DIM)
    if _trace:
        kernel.last_result = res
    return out

